# revision 12
# baseline (speedup 1.0000x reference)
"""Deformable-attention-3D Trainium2 kernel (v3, blocked-partition fp16 MAC).

Sharding: 8 cores = (batch b in {0,1}) x (query-block j in {0..3}).
Each core handles NB=8192 queries of one batch; inputs are host-sliced
per core (full x channel-major slab + halo), outputs host-concatenated.
No collectives.

The bilinear gather is 25 free-axis-offset FMAs (cells sy,sx in [-2,2]^2)
weighted by a tent-product weight grid
    w_cell(n,h) = sum_p attn[n,h,p] * relu(1-|off_y-sy|) * relu(1-|off_x-sx|)
with x-border masks and zeroed V halo for y/batch borders.

v3 layout: the MAC runs on tiles with partitions = (nb:4, h:8, dq:4) (nb =
512-query block of the superchunk, dq = dh/8 quarter) and free = (dd:8,
nr:512 + 2*132 halo).  In this layout the per-(head,block) cell weight
w25[(nb,h,dq), nr] multiplies V via a stride-0 broadcast AP over dd — no
dh replication on the PE and no big PSUM->SBUF weight-grid copies.  The
p-sum selector matmul writes w25 into packed PSUM rows directly (rows
64:128 via a zero-padded 64-row accumulating pair, since matmul dst base
must be 0/32/64); one small Act cast per cell moves it to fp16 SBUF.  V is
rearranged per superchunk into the blocked halo'd layout by SBUF->SBUF DMA
(1552-byte runs), and the fp16 accumulator is DMA-unshuffled back to
(hd, n) for the output projection.  All MAC tensor_tensor ops are all-SBUF
packed fp16 -> DVE 2x_1p mode; a tunable subset of accumulate-adds runs on
Pool.
"""
import os
import numpy as np
from contextlib import ExitStack

import concourse.bass as bass
import concourse.mybir as mybir
import concourse.tile as tile

F32 = mybir.dt.float32
F32R = mybir.dt.float32r
F16 = mybir.dt.float16
AF = mybir.ActivationFunctionType
OP = mybir.AluOpType

# problem constants
B, C, Z, HH, WW = 2, 256, 16, 32, 64
H, W = HH * Z, WW          # 512, 64
N = H * W                  # 32768 queries per batch
HEADS, P, DH = 8, 4, 32
NB = N // 4                # 8192 queries per core
HALO = 192                 # V halo each side (needs >= 130)
XEXT = NB + 2 * HALO       # 8576
NSC = 4                    # superchunks per core
SC = NB // NSC             # 2048
FC = 512                   # matmul moving-dim chunk / nb block size
HC = 132                   # per-block halo in the blocked V layout (>= 130)
NRX = FC + 2 * HC          # 776
SYS = (-2, -1, 0, 1, 2)
SXS = (-2, -1, 0, 1, 2)

# tunable: cells (index 0..24 in (sy,sx) order) whose accumulate-add runs
# on Pool instead of DVE.
POOL_MULT_SLOT = frozenset((1, 3))  # per-group cell slots whose mult -> Pool

_cache = {}


def _consts():
    """Host-computed constant tensors shared by all cores."""
    # p-sum + dq-replication selector: maps (h,p) rows -> (h', dq) cols.
    s8 = np.zeros((32, 32), np.float16)
    for h in range(8):
        for p in range(P):
            for dq in range(4):
                s8[h * 4 + p, h * 4 + dq] = 1.0
    # SELX [128, 160]: rows tiled x4 (r-blocks).  cols 0:32 = 32-row variant
    # (nb=0 at dst base 0, nb=1 at 32); cols 32:96 = 64-row pair A (nb=2 ->
    # out rows 64:96, rows 96:128 of lhs zero); cols 96:160 = pair B (nb=3
    # -> out rows 96:128).
    selx = np.zeros((128, 160), np.float16)
    for r in range(4):
        selx[32 * r:32 * (r + 1), 0:32] = s8
    selx[64:96, 32:64] = s8
    selx[96:128, 128:160] = s8
    # softmax p-sum selector, block-diagonal over the 4 r-blocks
    selp = np.zeros((32, 32), np.float16)
    for h in range(8):
        selp[h * 4:(h + 1) * 4, h * 4:(h + 1) * 4] = 1.0
    selp128 = np.zeros((128, 128), np.float16)
    for r in range(4):
        selp128[32 * r:32 * (r + 1), 32 * r:32 * (r + 1)] = selp
    # x-border masks for sx in (-2,-1,1,2): [128, 4*FC]
    xm = np.zeros((128, 4 * FC), np.float16)
    for k, sx in enumerate((-2, -1, 1, 2)):
        i = np.arange(FC)
        valid = ((i % W) + sx >= 0) & ((i % W) + sx < W)
        xm[:, k * FC:(k + 1) * FC] = valid.astype(np.float16)[None, :]
    return selx, selp128, xm


def build_program(reps=1):
    key = ("nc", reps)
    if key in _cache:
        return _cache[key]
    nc = bass.Bass()
    d = {}
    d["xT"] = nc.dram_tensor("xT", [C, XEXT], F32, kind="ExternalInput").ap()
    d["wv"] = nc.dram_tensor("wv", [C, C], F32, kind="ExternalInput").ap()
    d["woa"] = nc.dram_tensor("woa", [C, 224], F16, kind="ExternalInput").ap()
    d["wo"] = nc.dram_tensor("wo", [C, C], F16, kind="ExternalInput").ap()
    d["bval"] = nc.dram_tensor("bval", [C, 1], F32, kind="ExternalInput").ap()
    d["boa"] = nc.dram_tensor("boa", [3 * 128, 1], F32, kind="ExternalInput").ap()
    d["bout"] = nc.dram_tensor("bout", [C, 1], F32, kind="ExternalInput").ap()
    d["selx"] = nc.dram_tensor("selx", [128, 160], F16, kind="ExternalInput").ap()
    d["selp"] = nc.dram_tensor("selp", [128, 128], F16, kind="ExternalInput").ap()
    d["xmask"] = nc.dram_tensor("xmask", [128, 4 * FC], F16, kind="ExternalInput").ap()
    d["vmask"] = nc.dram_tensor("vmask", [128, 2 * HALO], F16, kind="ExternalInput").ap()
    d["cb"] = nc.dram_tensor("cb", [128, 5], F32, kind="ExternalInput").ap()
    d["out"] = nc.dram_tensor("out", [C, NB], F32, kind="ExternalOutput").ap()
    with tile.TileContext(nc) as tc, ExitStack() as ctx, \
            nc.allow_low_precision(reason="fp16 MAC validated vs reference"):
        if reps == 1:
            _kernel_body(ctx, tc, d)
        else:
            with tc.For_i(0, reps, 1):
                _kernel_body(ctx, tc, d)
    _legalize_waits(nc)
    _cache[key] = nc
    return nc


def _legalize_waits(nc):
    """This toolchain's walrus rejects >1 sem-wait on HW-decoded structs
    (fp32-family matmuls, drains) and has small caps elsewhere. Split excess
    waits onto standalone EventSemaphore instructions placed just before the
    offender on the same engine — semantically identical (waits still
    happen-before, same order)."""
    nid = 0
    for f in nc.m.functions:
        for bb in f.blocks:
            insts = bb.instructions
            i = 0
            while i < len(insts):
                inst = insts[i]
                si = inst.sync_info
                waits = list(si.on_wait) if (si and si.on_wait) else []
                limit = 1
                if len(waits) > limit:
                    keep = waits[len(waits) - limit:]
                    excess = waits[:len(waits) - limit]
                    inst.sync_info = mybir.SyncInfo(
                        on_wait=keep, on_update=list(si.on_update or []))
                    for w in excess:
                        ws = mybir.InstEventSemaphore(
                            name=f"WSPLIT-{nid}", ins=[], outs=[],
                            sync_info=mybir.SyncInfo(on_wait=[w], on_update=[]))
                        nid += 1
                        ws.engine = inst.engine
                        nc.register_instruction(ws, overwrite=True)
                        insts.insert(i, ws)
                        i += 1
                i += 1


def _kernel_body(ctx, tc, d):
    nc = tc.nc
    const = ctx.enter_context(tc.tile_pool(name="const", bufs=1))
    vpool = ctx.enter_context(tc.tile_pool(name="vpool", bufs=1))
    v2pool = ctx.enter_context(tc.tile_pool(name="v2pool", bufs=2))
    xin = ctx.enter_context(tc.tile_pool(name="xin", bufs=2))
    xin1 = ctx.enter_context(tc.tile_pool(name="xin1", bufs=1))
    build = ctx.enter_context(tc.tile_pool(name="build", bufs=1))
    wgt = ctx.enter_context(tc.tile_pool(name="wgt", bufs=2))
    macp = ctx.enter_context(tc.tile_pool(name="macp", bufs=2))
    accp = ctx.enter_context(tc.tile_pool(name="accp", bufs=1))
    a2pool = ctx.enter_context(tc.tile_pool(name="a2pool", bufs=1))
    outp = ctx.enter_context(tc.tile_pool(name="outp", bufs=2))
    # PSUM budget (8 banks): psW 2x[128,512]=2, psQ 3x[128,512]=3,
    # psS 1x[128,512]=1, psC 2x[128,512]=2.
    psW = ctx.enter_context(tc.tile_pool(name="psW", bufs=2, space="PSUM"))
    psQ = ctx.enter_context(tc.tile_pool(name="psQ", bufs=1, space="PSUM"))
    psS = ctx.enter_context(tc.tile_pool(name="psS", bufs=1, space="PSUM"))
    psC = ctx.enter_context(tc.tile_pool(name="psC", bufs=2, space="PSUM"))

    # ---- constants to SBUF
    wv_t = [const.tile([128, C], F32R, tag=f"wv{k}", name=f"wv{k}") for k in range(2)]
    wo_t = [const.tile([128, C], F16, tag=f"wo{k}", name=f"wo{k}") for k in range(2)]
    woa_t = [const.tile([128, 224], F16, tag=f"woa{k}", name=f"woa{k}") for k in range(2)]
    selx_t = const.tile([128, 160], F16, tag="selx", name="selx")
    selp_t = const.tile([128, 128], F16, tag="selp", name="selp")
    xm_t = const.tile([128, 4 * FC], F16, tag="xm", name="xm")
    vm_t = const.tile([128, 2 * HALO], F16, tag="vm", name="vm")
    bval_t = [const.tile([128, 1], F32, tag=f"bv{g}", name=f"bv{g}") for g in range(2)]
    boa_t = [const.tile([128, 1], F32, tag=f"boa{q}", name=f"boa{q}") for q in range(3)]
    bout_t = [const.tile([128, 1], F32, tag=f"bo{m}", name=f"bo{m}") for m in range(2)]
    cb_t = const.tile([128, 5], F32, tag="cb", name="cb")
    for k in range(2):
        nc.sync.dma_start(wv_t[k][:], d["wv"][k * 128:(k + 1) * 128, :].bitcast(F32R))
        nc.sync.dma_start(wo_t[k][:], d["wo"][k * 128:(k + 1) * 128, :])
        nc.sync.dma_start(woa_t[k][:], d["woa"][k * 128:(k + 1) * 128, :])
        nc.sync.dma_start(bval_t[k][:], d["bval"][k * 128:(k + 1) * 128, :])
        nc.sync.dma_start(bout_t[k][:], d["bout"][k * 128:(k + 1) * 128, :])
    nc.sync.dma_start(selx_t[:], d["selx"][:])
    nc.sync.dma_start(selp_t[:], d["selp"][:])
    nc.sync.dma_start(xm_t[:], d["xmask"][:])
    nc.sync.dma_start(vm_t[:], d["vmask"][:])
    nc.sync.dma_start(cb_t[:], d["cb"][:])
    for q in range(3):
        nc.sync.dma_start(boa_t[q][:], d["boa"][q * 128:(q + 1) * 128, :])

    # ---- phase 1: value projection V[g][128=(h4,d32), XEXT], fp16
    V = [vpool.tile([128, XEXT], F16, tag=f"V{g}", name=f"V{g}") for g in range(2)]
    CH = XEXT // 8  # 1072
    for r in range(8):
        xt = [xin1.tile([128, CH], F32R, tag=f"x1_{k}", name=f"x1_{k}") for k in range(2)]
        for k in range(2):
            nc.sync.dma_start(xt[k][:], d["xT"][k * 128:(k + 1) * 128,
                                                r * CH:(r + 1) * CH].bitcast(F32R))
        for g in range(2):
            nf = 0
            while nf < CH:
                f = min(FC, CH - nf)
                pv = psW.tile([128, FC], F32, tag="w", name="pv")
                for k in range(2):
                    nc.tensor.matmul(
                        pv[:, :f],
                        wv_t[k][:, g * 128:(g + 1) * 128],
                        xt[k][:, nf:nf + f],
                        start=(k == 0), stop=(k == 1))
                nc.scalar.activation(V[g][:, r * CH + nf:r * CH + nf + f],
                                     pv[:, :f], AF.Identity,
                                     bias=bval_t[g][:], scale=1.0)
                nf += f
    # zero out-of-batch halo (vmask is all-ones for interior cores)
    for g in range(2):
        nc.vector.tensor_tensor(V[g][:, 0:HALO], V[g][:, 0:HALO],
                                vm_t[:, 0:HALO], OP.mult)
        nc.vector.tensor_tensor(V[g][:, XEXT - HALO:XEXT],
                                V[g][:, XEXT - HALO:XEXT],
                                vm_t[:, HALO:2 * HALO], OP.mult)

    # ---- phase 2: per superchunk
    for sc in range(NSC):
        q0 = sc * SC

        # blocked V rearrange: V2[(nb,h,dq) = nb*32+h*4+dq, (dd:8, nr:NRX)]
        # fp16 with per-block halo HC, from V[g] partition (h%4)*32+dq*8+dd.
        V2 = v2pool.tile([128, 8, NRX], F16, tag="V2", name="V2")
        for nb in range(4):
            for g in range(2):
                src = V[g][:, HALO + q0 + nb * FC - HC:
                           HALO + q0 + nb * FC - HC + NRX]
                dst = V2[nb * 32 + 16 * g:nb * 32 + 16 * g + 16, :, :]
                nc.sync.dma_start(dst, src)

        # offsets/attn projections straight into packed (4r x 32hp) PSUM rows.
        pq3 = [psQ.tile([128, FC], F32, tag=f"q{q}", name=f"q{q}") for q in range(3)]
        for r in range(NSC):
            xt = [xin.tile([128, FC], F32, tag=f"x2_{k}", name=f"x2_{k}")
                  for k in range(2)]
            xt16 = [xin.tile([128, FC], F16, tag=f"x16_{k}", name=f"x16_{k}")
                    for k in range(2)]
            for k in range(2):
                nc.sync.dma_start(
                    xt[k][:],
                    d["xT"][k * 128:(k + 1) * 128,
                            HALO + q0 + r * FC:HALO + q0 + (r + 1) * FC])
                nc.scalar.activation(xt16[k][:], xt[k][:], AF.Identity)
            for q in range(3):
                if r < 2:
                    for k in range(2):
                        nc.tensor.matmul(
                            pq3[q][32 * r:32 * (r + 1), :],
                            woa_t[k][:, 32 + 64 * q:64 + 64 * q],
                            xt16[k][:],
                            start=(k == 0), stop=(k == 1))
                elif r == 2:
                    for k in range(2):
                        nc.tensor.matmul(
                            pq3[q][64:128, :],
                            woa_t[k][:, 32 + 64 * q:96 + 64 * q],
                            xt16[k][:],
                            start=(k == 0), stop=False)
                else:
                    for k in range(2):
                        nc.tensor.matmul(
                            pq3[q][64:128, :],
                            woa_t[k][:, 64 * q:64 + 64 * q],
                            xt16[k][:],
                            start=False, stop=(k == 1))

        # biases + softmax (fp16 outputs)
        oyp = build.tile([128, FC], F32, tag="oyp", name="oyp")
        oxp = build.tile([128, FC], F32, tag="oxp", name="oxp")
        ex = build.tile([128, FC], F16, tag="ex", name="ex")
        rc = build.tile([128, FC], F16, tag="rc", name="rc")
        atp = build.tile([128, FC], F16, tag="atp", name="atp")
        nc.scalar.activation(oyp[:], pq3[0][:], AF.Identity, bias=boa_t[0][:],
                             scale=1.0)
        nc.scalar.activation(oxp[:], pq3[1][:], AF.Identity, bias=boa_t[1][:],
                             scale=1.0)
        nc.scalar.activation(ex[:], pq3[2][:], AF.Exp, bias=boa_t[2][:],
                             scale=1.0)
        pss = psS.tile([128, FC], F32, tag="ps", name="ps")
        nc.tensor.matmul(pss[:], selp_t[:], ex[:], start=True, stop=True)
        nc.vector.reciprocal(rc[:], pss[:])
        nc.vector.tensor_tensor(atp[:], ex[:], rc[:], OP.mult)

        # tent weights (fp16): AYA[sy] = attn * relu(1-|off_y - sy|),
        #                      AXM[sx] = relu(1-|off_x - sx|) * xmask
        aya, axm = {}, {}
        for sy in SYS:
            u = build.tile([128, FC], F32, tag="u", name="u", bufs=2)
            nc.scalar.activation(u[:], oyp[:], AF.Abs,
                                 bias=cb_t[:, sy + 2:sy + 3], scale=1.0)
            t = build.tile([128, FC], F16, tag=f"aya{sy}", name=f"aya{sy}")
            nc.scalar.activation(t[:], u[:], AF.Relu, bias=1.0, scale=-1.0)
            nc.vector.tensor_tensor(t[:], t[:], atp[:], OP.mult)
            aya[sy] = t
        xmi = {-2: 0, -1: 1, 1: 2, 2: 3}
        for sx in SXS:
            u = build.tile([128, FC], F32, tag="u", name="u", bufs=2)
            nc.scalar.activation(u[:], oxp[:], AF.Abs,
                                 bias=cb_t[:, sx + 2:sx + 3], scale=1.0)
            t = build.tile([128, FC], F16, tag=f"axm{sx}", name=f"axm{sx}")
            nc.scalar.activation(t[:], u[:], AF.Relu, bias=1.0, scale=-1.0)
            if sx != 0:
                k = xmi[sx]
                nc.vector.tensor_tensor(t[:], t[:],
                                        xm_t[:, k * FC:(k + 1) * FC], OP.mult)
            axm[sx] = t

        # cell loop: per-cell p-summed weight w25[(nb,h,dq), nr] via one
        # 4-matmul selector group into packed PSUM, small Act fp16 cast,
        # then broadcast-AP mult + tree accumulate (5 independent per-sy
        # group chains + 3-level final merge; POOL_MULT cells' multiplies
        # run on Pool, everything else on DVE).
        gacc = {}
        for sy in SYS:
            # per sy-group: Pool-assigned cells' multiplies issue first and
            # their accumulates fold in last, so the DVE sub-chain overlaps
            # Pool latency instead of blocking on it.
            slot = -1
            dve_cells, pool_cells = [], []
            for sx in SXS:
                slot += 1
                (pool_cells if slot in POOL_MULT_SLOT else dve_cells).append(sx)
            cw = {}
            for sx in SXS:
                pr = wgt.tile([128, FC], F16, tag="pr", name="pr", bufs=4)
                nc.vector.tensor_tensor(pr[:], aya[sy][:], axm[sx][:], OP.mult)
                pw = psC.tile([128, FC], F32, tag="c", name="pw")
                for nb in range(2):
                    nc.tensor.matmul(
                        pw[32 * nb:32 * (nb + 1), :],
                        selx_t[32 * nb:32 * (nb + 1), 0:32],
                        pr[32 * nb:32 * (nb + 1), :],
                        start=True, stop=True)
                nc.tensor.matmul(
                    pw[64:128, :], selx_t[64:128, 32:96], pr[64:128, :],
                    start=True, stop=False)
                nc.tensor.matmul(
                    pw[64:128, :], selx_t[64:128, 96:160], pr[64:128, :],
                    start=False, stop=True)
                w25 = wgt.tile([128, FC], F16, tag="w25", name="w25", bufs=6)
                nc.scalar.activation(w25[:], pw[:], AF.Identity)
                cw[sx] = w25
            ptmp = {}
            for sx in pool_cells:
                wb = cw[sx][:].unsqueeze(1).broadcast_to([128, 8, FC])
                vsl = V2[:, :, HC + sy * W + sx:HC + sy * W + sx + FC]
                tp_ = macp.tile([128, 8, FC], F16, tag="tmpp", name="tmpp",
                                bufs=2)
                nc.gpsimd.tensor_tensor(tp_[:], vsl, wb, OP.mult)
                ptmp[sx] = tp_
            first = True
            for sx in dve_cells:
                wb = cw[sx][:].unsqueeze(1).broadcast_to([128, 8, FC])
                vsl = V2[:, :, HC + sy * W + sx:HC + sy * W + sx + FC]
                if first:
                    ga = accp.tile([128, 8, FC], F16, tag=f"ga{sy}",
                                   name=f"ga{sy}")
                    gacc[sy] = ga
                    nc.vector.tensor_tensor(ga[:], vsl, wb, OP.mult)
                    first = False
                else:
                    tmp = macp.tile([128, 8, FC], F16, tag="tmp", name="tmp")
                    nc.vector.tensor_tensor(tmp[:], vsl, wb, OP.mult)
                    nc.vector.tensor_tensor(gacc[sy][:], gacc[sy][:], tmp[:],
                                            OP.add)
            for sx in pool_cells:
                nc.vector.tensor_tensor(gacc[sy][:], gacc[sy][:],
                                        ptmp[sx][:], OP.add)
        # final merge: ((g[-2]+g[-1]) + (g[1]+g[2])) + g[0] -> gacc[-2]
        nc.vector.tensor_tensor(gacc[-2][:], gacc[-2][:], gacc[-1][:], OP.add)
        nc.vector.tensor_tensor(gacc[1][:], gacc[1][:], gacc[2][:], OP.add)
        nc.vector.tensor_tensor(gacc[0][:], gacc[0][:], gacc[1][:], OP.add)
        nc.vector.tensor_tensor(gacc[-2][:], gacc[-2][:], gacc[0][:], OP.add)
        acc = gacc[-2]

        # unshuffle acc [(nb,h,dq), (dd, nr)] -> A2[g][(h%4)*32+d, n] fp16
        A2 = [a2pool.tile([128, SC], F16, tag=f"A2{g}", name=f"A2{g}")
              for g in range(2)]
        for nb in range(4):
            for g in range(2):
                src = acc[nb * 32 + 16 * g:nb * 32 + 16 * g + 16, :, :]
                dst = A2[g][:, nb * FC:(nb + 1) * FC]
                nc.sync.dma_start(dst, src)

        # output projection: out[c, n] = wo.T @ head_out + bout
        for mc in range(2):
            for fc in range(NSC):
                po = psW.tile([128, FC], F32, tag="w", name="po")
                for g in range(2):
                    nc.tensor.matmul(
                        po[:],
                        wo_t[g][:, mc * 128:(mc + 1) * 128],
                        A2[g][:, fc * FC:(fc + 1) * FC],
                        start=(g == 0), stop=(g == 1))
                ot = outp.tile([128, FC], F32, tag="ot", name="ot")
                nc.scalar.activation(ot[:], po[:], AF.Identity,
                                     bias=bout_t[mc][:], scale=1.0)
                nc.sync.dma_start(
                    d["out"][mc * 128:(mc + 1) * 128,
                             q0 + fc * FC:q0 + (fc + 1) * FC], ot[:])


def prep_inputs(x, w_off, b_off, w_attn, b_attn, w_val, b_val, w_out, b_out):
    """Host-side sharding: returns list of 8 per-core input dicts."""
    selx, selp, xmask = _consts()
    # woa layout [C, 224]: 32 leading zero cols, then per q in (off_y, off_x,
    # attn): w_q at cols 32+64q:64+64q, zeros at 64+64q:96+64q.
    woa = np.zeros((C, 224), np.float16)
    wq = [w_off[1::2], w_off[0::2], w_attn]
    bq = [b_off[1::2], b_off[0::2], b_attn]
    for q in range(3):
        woa[:, 32 + 64 * q:64 + 64 * q] = wq[q].T
    boa = np.zeros((3 * 128, 1), np.float32)
    for q in range(3):
        boa[q * 128:(q + 1) * 128, 0] = np.tile(bq[q], 4)
    shared = {
        "wv": np.ascontiguousarray(w_val.T).astype(np.float32),
        "woa": woa,
        "wo": np.ascontiguousarray(w_out.T).astype(np.float16),
        "bval": np.ascontiguousarray(b_val[:, None]).astype(np.float32),
        "boa": boa,
        "bout": np.ascontiguousarray(b_out[:, None]).astype(np.float32),
        "selx": selx, "selp": selp, "xmask": xmask,
        "cb": np.tile(np.array([2.0, 1.0, 0.0, -1.0, -2.0], np.float32),
                      (128, 1)),
    }
    in_maps = []
    for core in range(8):
        b, j = divmod(core, 4)
        n0 = j * NB
        xb = x[b].reshape(C, N)
        xt = np.zeros((C, XEXT), np.float32)
        lo, hi = n0 - HALO, n0 + NB + HALO
        clo, chi = max(lo, 0), min(hi, N)
        xt[:, clo - lo:chi - lo] = xb[:, clo:chi]
        vm = np.ones((128, 2 * HALO), np.float16)
        if j == 0:
            vm[:, :HALO] = 0.0
        if j == 3:
            vm[:, HALO:] = 0.0
        m = dict(shared)
        m["xT"] = xt
        m["vmask"] = vm
        in_maps.append(m)
    return in_maps


def assemble(results):
    out = np.zeros((B, C, N), np.float32)
    for core in range(8):
        b, j = divmod(core, 4)
        out[b, :, j * NB:(j + 1) * NB] = results[core]["out"]
    return out.reshape(B, C, Z, HH, WW)


last_exec_ns = None


def kernel(**inputs):
    global last_exec_ns
    from concourse.bass_utils import run_bass_kernel_spmd
    nc = build_program()
    in_maps = prep_inputs(**inputs)
    res = run_bass_kernel_spmd(nc, in_maps, list(range(8)))
    last_exec_ns = res.exec_time_ns
    return assemble(res.results)


# revision 13
# speedup vs baseline: 1.5752x; 1.5752x over previous
"""Deformable-attention-3D Trainium2 kernel (v2, fp16 MAC).

Sharding: 8 cores = (batch b in {0,1}) x (query-block j in {0..3}).
Each core handles NB=8192 queries of one batch; inputs are host-sliced
per core (full x channel-major slab + halo), outputs host-concatenated.
No collectives.

Per-core layout: partition = (head, dh) for value/output, free axis = query n.
The bilinear gather becomes 25 free-axis-offset FMAs (cells sy,sx in
[-2,2]^2) weighted by a tent-product weight grid:
    w_cell(n,h) = sum_p attn[n,h,p] * relu(1-|off_y-sy|) * relu(1-|off_x-sx|)
with x-border masks and zeroed V halo for y/batch borders.

v2 changes vs v1:
 - value tensor, tent weights, replicated weight grids, tmp and acc all
   fp16 -> DVE tensor_tensor runs in 2x_1p mode (2x).
 - Act engine evacuates the PE-replicated weight grid PSUM->SBUF fp16,
   so the big MAC multiplies are all-SBUF fp16.
 - offsets/attn projections matmul directly into packed partition rows
   of [128,*] PSUM tiles (no SBUF->SBUF repack DMAs); rows 96:128 are
   written via a 64-row accumulating matmul pair at base 64.
 - MAC mult/add run as single [128,2048]-wide ops; a tunable subset of
   cells' accumulate-adds runs on the Pool engine.
"""
import os
import numpy as np
from contextlib import ExitStack

import concourse.bass as bass
import concourse.mybir as mybir
import concourse.tile as tile

F32 = mybir.dt.float32
F32R = mybir.dt.float32r
F16 = mybir.dt.float16
AF = mybir.ActivationFunctionType
OP = mybir.AluOpType

# problem constants
B, C, Z, HH, WW = 2, 256, 16, 32, 64
H, W = HH * Z, WW          # 512, 64
N = H * W                  # 32768 queries per batch
HEADS, P, DH = 8, 4, 32
NB = N // 4                # 8192 queries per core
HALO = 192                 # V halo each side (needs >= 130)
XEXT = NB + 2 * HALO       # 8576
NSC = 4                    # superchunks per core
SC = NB // NSC             # 2048
FC = 512                   # matmul moving-dim chunk
SYS = (-2, -1, 0, 1, 2)
SXS = (-2, -1, 0, 1, 2)

# tunable: sy-groups whose accumulate-add chains run on Pool instead of DVE.
POOL_SYS = frozenset((1, 2))

_cache = {}


def _consts():
    """Host-computed constant tensors shared by all cores."""
    # selector: maps (h,p) rows -> (h', d) cols for head-group g; stacked x4
    # for the 4 base partitions (r-major packing of n-subchunks).
    # Layout [128, 256]: cols 0:128 = sel32 stacked x4 (slices r=0,1,2 used at
    # bases 0/32/64); cols 128:256 = K=64 variant for r=3 at base 64 (rows
    # 64:96 zero so the r2 partitions contribute nothing, rows 96:128 = sel32).
    sels = []
    for g in range(2):
        s = np.zeros((32, 128), np.float16)
        for h in range(8):
            for p in range(P):
                for hh in range(4):
                    if h == 4 * g + hh:
                        s[h * 4 + p, hh * 32:(hh + 1) * 32] = 1.0
        full = np.zeros((128, 256), np.float16)
        full[:, :128] = np.tile(s, (4, 1))
        full[96:128, 128:256] = s
        sels.append(full)
    # softmax p-sum selector, block-diagonal over the 4 r-blocks:
    # [128, 128], out row (r,h,p) = sum_{p'} in row (r,h,p')
    selp = np.zeros((32, 32), np.float16)
    for h in range(8):
        selp[h * 4:(h + 1) * 4, h * 4:(h + 1) * 4] = 1.0
    selp128 = np.zeros((128, 128), np.float16)
    for r in range(4):
        selp128[32 * r:32 * (r + 1), 32 * r:32 * (r + 1)] = selp
    # x-border masks for sx in (-2,-1,1,2): [128, 4*FC]
    xm = np.zeros((128, 4 * FC), np.float16)
    for k, sx in enumerate((-2, -1, 1, 2)):
        i = np.arange(FC)
        valid = ((i % W) + sx >= 0) & ((i % W) + sx < W)
        xm[:, k * FC:(k + 1) * FC] = valid.astype(np.float16)[None, :]
    return sels[0], sels[1], selp128, xm


def build_program(reps=1):
    key = ("nc", reps)
    if key in _cache:
        return _cache[key]
    nc = bass.Bass()
    d = {}
    d["xT"] = nc.dram_tensor("xT", [C, XEXT], F32, kind="ExternalInput").ap()
    d["wv"] = nc.dram_tensor("wv", [C, C], F32, kind="ExternalInput").ap()
    d["woa"] = nc.dram_tensor("woa", [C, 224], F16, kind="ExternalInput").ap()
    d["wo"] = nc.dram_tensor("wo", [C, C], F16, kind="ExternalInput").ap()
    d["bval"] = nc.dram_tensor("bval", [C, 1], F32, kind="ExternalInput").ap()
    d["boa"] = nc.dram_tensor("boa", [3 * 128, 1], F32, kind="ExternalInput").ap()
    d["bout"] = nc.dram_tensor("bout", [C, 1], F32, kind="ExternalInput").ap()
    d["sel0"] = nc.dram_tensor("sel0", [128, 256], F16, kind="ExternalInput").ap()
    d["sel1"] = nc.dram_tensor("sel1", [128, 256], F16, kind="ExternalInput").ap()
    d["selp"] = nc.dram_tensor("selp", [128, 128], F16, kind="ExternalInput").ap()
    d["xmask"] = nc.dram_tensor("xmask", [128, 4 * FC], F16, kind="ExternalInput").ap()
    d["vmask"] = nc.dram_tensor("vmask", [128, 2 * HALO], F16, kind="ExternalInput").ap()
    d["cb"] = nc.dram_tensor("cb", [128, 5], F32, kind="ExternalInput").ap()
    d["out"] = nc.dram_tensor("out", [C, NB], F32, kind="ExternalOutput").ap()
    with tile.TileContext(nc) as tc, ExitStack() as ctx, \
            nc.allow_low_precision(reason="fp16 MAC validated vs reference"):
        if reps == 1:
            _kernel_body(ctx, tc, d)
        else:
            with tc.For_i(0, reps, 1):
                _kernel_body(ctx, tc, d)
    _legalize_waits(nc)
    _cache[key] = nc
    return nc


def _legalize_waits(nc):
    """This toolchain's walrus rejects >1 sem-wait on HW-decoded structs
    (fp32-family matmuls, drains) and has small caps elsewhere. Split excess
    waits onto standalone EventSemaphore instructions placed just before the
    offender on the same engine — semantically identical (waits still
    happen-before, same order)."""
    PE = mybir.EngineType.PE
    nid = 0
    for f in nc.m.functions:
        for bb in f.blocks:
            insts = bb.instructions
            i = 0
            while i < len(insts):
                inst = insts[i]
                si = inst.sync_info
                waits = list(si.on_wait) if (si and si.on_wait) else []
                cls = type(inst).__name__
                limit = 1
                if len(waits) > limit:
                    keep = waits[len(waits) - limit:]
                    excess = waits[:len(waits) - limit]
                    inst.sync_info = mybir.SyncInfo(
                        on_wait=keep, on_update=list(si.on_update or []))
                    for w in excess:
                        ws = mybir.InstEventSemaphore(
                            name=f"WSPLIT-{nid}", ins=[], outs=[],
                            sync_info=mybir.SyncInfo(on_wait=[w], on_update=[]))
                        nid += 1
                        ws.engine = inst.engine
                        nc.register_instruction(ws, overwrite=True)
                        insts.insert(i, ws)
                        i += 1
                i += 1


def _kernel_body(ctx, tc, d):
    nc = tc.nc
    const = ctx.enter_context(tc.tile_pool(name="const", bufs=1))
    vpool = ctx.enter_context(tc.tile_pool(name="vpool", bufs=1))
    xin = ctx.enter_context(tc.tile_pool(name="xin", bufs=2))
    small = ctx.enter_context(tc.tile_pool(name="small", bufs=2))
    build = ctx.enter_context(tc.tile_pool(name="build", bufs=1))
    wgt = ctx.enter_context(tc.tile_pool(name="wgt", bufs=2))
    macp = ctx.enter_context(tc.tile_pool(name="macp", bufs=3))
    accp = ctx.enter_context(tc.tile_pool(name="accp", bufs=1))
    outp = ctx.enter_context(tc.tile_pool(name="outp", bufs=2))
    # PSUM budget (8 banks): psW 2 bufs x [128,1024] = 4, psQ 3 x [128,512]
    # = 3, psS 1 x [128,512] = 1.
    psW = ctx.enter_context(tc.tile_pool(name="psW", bufs=2, space="PSUM"))
    psQ = ctx.enter_context(tc.tile_pool(name="psQ", bufs=1, space="PSUM"))
    psS = ctx.enter_context(tc.tile_pool(name="psS", bufs=1, space="PSUM"))

    # ---- constants to SBUF
    wv_t = [const.tile([128, C], F32R, tag=f"wv{k}", name=f"wv{k}") for k in range(2)]
    wo_t = [const.tile([128, C], F16, tag=f"wo{k}", name=f"wo{k}") for k in range(2)]
    woa_t = [const.tile([128, 224], F16, tag=f"woa{k}", name=f"woa{k}") for k in range(2)]
    sel_t = [const.tile([128, 256], F16, tag=f"sel{g}", name=f"sel{g}") for g in range(2)]
    selp_t = const.tile([128, 128], F16, tag="selp", name="selp")
    xm_t = const.tile([128, 4 * FC], F16, tag="xm", name="xm")
    vm_t = const.tile([128, 2 * HALO], F16, tag="vm", name="vm")
    bval_t = [const.tile([128, 1], F32, tag=f"bv{g}", name=f"bv{g}") for g in range(2)]
    boa_t = [const.tile([128, 1], F32, tag=f"boa{q}", name=f"boa{q}") for q in range(3)]
    bout_t = [const.tile([128, 1], F32, tag=f"bo{m}", name=f"bo{m}") for m in range(2)]
    cb_t = const.tile([128, 5], F32, tag="cb", name="cb")
    for k in range(2):
        nc.sync.dma_start(wv_t[k][:], d["wv"][k * 128:(k + 1) * 128, :].bitcast(F32R))
        nc.sync.dma_start(wo_t[k][:], d["wo"][k * 128:(k + 1) * 128, :])
        nc.sync.dma_start(woa_t[k][:], d["woa"][k * 128:(k + 1) * 128, :])
        nc.sync.dma_start(bval_t[k][:], d["bval"][k * 128:(k + 1) * 128, :])
        nc.sync.dma_start(bout_t[k][:], d["bout"][k * 128:(k + 1) * 128, :])
    nc.sync.dma_start(sel_t[0][:], d["sel0"][:])
    nc.sync.dma_start(sel_t[1][:], d["sel1"][:])
    nc.sync.dma_start(selp_t[:], d["selp"][:])
    nc.sync.dma_start(xm_t[:], d["xmask"][:])
    nc.sync.dma_start(vm_t[:], d["vmask"][:])
    nc.sync.dma_start(cb_t[:], d["cb"][:])
    for q in range(3):
        nc.sync.dma_start(boa_t[q][:], d["boa"][q * 128:(q + 1) * 128, :])

    # ---- phase 1: value projection V[hg][128, XEXT], fp16, hd-major partitions
    V = [vpool.tile([128, XEXT], F16, tag=f"V{g}", name=f"V{g}") for g in range(2)]
    CH = XEXT // 8  # 1072
    for r in range(8):
        xt = [xin.tile([128, CH], F32R, tag=f"x1_{k}", name=f"x1_{k}") for k in range(2)]
        for k in range(2):
            nc.sync.dma_start(xt[k][:], d["xT"][k * 128:(k + 1) * 128,
                                                r * CH:(r + 1) * CH].bitcast(F32R))
        for g in range(2):
            nf = 0
            while nf < CH:
                f = min(FC, CH - nf)
                pv = psW.tile([128, 1024], F32, tag="w", name="pv")
                for k in range(2):
                    nc.tensor.matmul(
                        pv[:, :f],
                        wv_t[k][:, g * 128:(g + 1) * 128],
                        xt[k][:, nf:nf + f],
                        start=(k == 0), stop=(k == 1))
                nc.scalar.activation(V[g][:, r * CH + nf:r * CH + nf + f],
                                     pv[:, :f], AF.Identity,
                                     bias=bval_t[g][:], scale=1.0)
                nf += f
    # zero out-of-batch halo (vmask is all-ones for interior cores)
    for g in range(2):
        nc.vector.tensor_tensor(V[g][:, 0:HALO], V[g][:, 0:HALO],
                                vm_t[:, 0:HALO], OP.mult)
        nc.vector.tensor_tensor(V[g][:, XEXT - HALO:XEXT],
                                V[g][:, XEXT - HALO:XEXT],
                                vm_t[:, HALO:2 * HALO], OP.mult)

    # ---- phase 2: per superchunk
    for sc in range(NSC):
        q0 = sc * SC

        # offsets/attn projections straight into packed (4r x 32hp) PSUM rows.
        # rows 96:128 can't be a matmul base, so r=2,3 are a 64-row pair at
        # base 64 with zero-padded lhs columns (woa layout: [C,224], q-block
        # at cols 32+64q:64+64q, zeros elsewhere).
        pq3 = [psQ.tile([128, FC], F32, tag=f"q{q}", name=f"q{q}") for q in range(3)]
        for r in range(NSC):
            xt = [xin.tile([128, FC], F32, tag=f"x2_{k}", name=f"x2_{k}")
                  for k in range(2)]
            xt16 = [xin.tile([128, FC], F16, tag=f"x16_{k}", name=f"x16_{k}")
                    for k in range(2)]
            for k in range(2):
                nc.sync.dma_start(
                    xt[k][:],
                    d["xT"][k * 128:(k + 1) * 128,
                            HALO + q0 + r * FC:HALO + q0 + (r + 1) * FC])
                nc.vector.tensor_scalar(xt16[k][:], xt[k][:], 1.0, None,
                                        OP.mult)
            for q in range(3):
                if r < 2:
                    for k in range(2):
                        nc.tensor.matmul(
                            pq3[q][32 * r:32 * (r + 1), :],
                            woa_t[k][:, 32 + 64 * q:64 + 64 * q],
                            xt16[k][:],
                            start=(k == 0), stop=(k == 1))
                elif r == 2:
                    for k in range(2):
                        nc.tensor.matmul(
                            pq3[q][64:128, :],
                            woa_t[k][:, 32 + 64 * q:96 + 64 * q],
                            xt16[k][:],
                            start=(k == 0), stop=False)
                else:
                    for k in range(2):
                        nc.tensor.matmul(
                            pq3[q][64:128, :],
                            woa_t[k][:, 64 * q:64 + 64 * q],
                            xt16[k][:],
                            start=False, stop=(k == 1))

        # biases + softmax (fp16 outputs)
        oyp = build.tile([128, FC], F32, tag="oyp", name="oyp")
        oxp = build.tile([128, FC], F32, tag="oxp", name="oxp")
        ex = build.tile([128, FC], F16, tag="ex", name="ex")
        rc = build.tile([128, FC], F16, tag="rc", name="rc")
        atp = build.tile([128, FC], F16, tag="atp", name="atp")
        nc.scalar.activation(oyp[:], pq3[0][:], AF.Identity, bias=boa_t[0][:],
                             scale=1.0)
        nc.scalar.activation(oxp[:], pq3[1][:], AF.Identity, bias=boa_t[1][:],
                             scale=1.0)
        nc.scalar.activation(ex[:], pq3[2][:], AF.Exp, bias=boa_t[2][:],
                             scale=1.0)
        pss = psS.tile([128, FC], F32, tag="ps", name="ps")
        nc.tensor.matmul(pss[:], selp_t[:], ex[:], start=True, stop=True)
        nc.vector.reciprocal(rc[:], pss[:])
        nc.vector.tensor_tensor(atp[:], ex[:], rc[:], OP.mult)

        # tent weights: AYA[sy] = attn * relu(1-|off_y - sy|),
        #               AXM[sx] = relu(1-|off_x - sx|) * xmask   (all fp16)
        aya, axm = {}, {}
        for sy in SYS:
            u = build.tile([128, FC], F32, tag="u", name="u", bufs=2)
            nc.scalar.activation(u[:], oyp[:], AF.Abs,
                                 bias=cb_t[:, sy + 2:sy + 3], scale=1.0)
            t = build.tile([128, FC], F16, tag=f"aya{sy}", name=f"aya{sy}")
            nc.scalar.activation(t[:], u[:], AF.Relu, bias=1.0, scale=-1.0)
            nc.vector.tensor_tensor(t[:], t[:], atp[:], OP.mult)
            aya[sy] = t
        xmi = {-2: 0, -1: 1, 1: 2, 2: 3}
        for sx in SXS:
            u = build.tile([128, FC], F32, tag="u", name="u", bufs=2)
            nc.scalar.activation(u[:], oxp[:], AF.Abs,
                                 bias=cb_t[:, sx + 2:sx + 3], scale=1.0)
            t = build.tile([128, FC], F16, tag=f"axm{sx}", name=f"axm{sx}")
            nc.scalar.activation(t[:], u[:], AF.Relu, bias=1.0, scale=-1.0)
            if sx != 0:
                k = xmi[sx]
                nc.vector.tensor_tensor(t[:], t[:],
                                        xm_t[:, k * FC:(k + 1) * FC], OP.mult)
            axm[sx] = t

        # cell loop: weight grid -> PE replication -> Act psum->sbuf fp16 ->
        # wide fp16 mult (DVE) + tree accumulate: one independent chain per
        # (sy-group, g); POOL_SYS groups' chains run on Pool (DVE produces
        # their tmps ahead); 4 final merge adds per g on DVE.
        gacc = {sy: {} for sy in SYS}
        for sy in SYS:
            on_pool = sy in POOL_SYS
            for sx in SXS:
                pr = wgt.tile([128, FC], F16, tag="pr", name="pr", bufs=3)
                nc.vector.tensor_tensor(pr[:], aya[sy][:], axm[sx][:], OP.mult)
                dlt = sy * W + sx
                vb = HALO + q0 + dlt
                first = (sx == SXS[0])
                for g in range(2):
                    wrep = wgt.tile([128, SC], F16, tag=f"wr{g}",
                                    name=f"wr{g}", bufs=2)
                    for hf in range(2):
                        pw = psW.tile([128, 1024], F32, tag="w", name="pw")
                        for rr in range(2):
                            r = 2 * hf + rr
                            if r < 3:
                                lhs = sel_t[g][32 * r:32 * (r + 1), 0:128]
                                rh = pr[32 * r:32 * (r + 1), :]
                            else:
                                lhs = sel_t[g][64:128, 128:256]
                                rh = pr[64:128, :]
                            nc.tensor.matmul(
                                pw[:, rr * FC:(rr + 1) * FC],
                                lhs, rh,
                                start=True, stop=True)
                        nc.scalar.activation(
                            wrep[:, hf * 1024:(hf + 1) * 1024], pw[:],
                            AF.Identity)
                    vs = V[g][:, vb:vb + SC]
                    if first:
                        ga = accp.tile([128, SC], F16, tag=f"ga{sy}_{g}",
                                       name=f"ga{sy}_{g}")
                        gacc[sy][g] = ga
                        nc.vector.tensor_tensor(ga[:], vs, wrep[:], OP.mult)
                    else:
                        tmp = macp.tile([128, SC], F16, tag="tmp", name="tmp",
                                        bufs=4)
                        nc.vector.tensor_tensor(tmp[:], vs, wrep[:], OP.mult)
                        if on_pool:
                            nc.gpsimd.tensor_tensor(gacc[sy][g][:],
                                                    gacc[sy][g][:], tmp[:],
                                                    OP.add)
                        else:
                            nc.vector.tensor_tensor(gacc[sy][g][:],
                                                    gacc[sy][g][:], tmp[:],
                                                    OP.add)
        # final merges per g (DVE): ((g[-2]+g[-1]) + g[0]) + (g[1] + g[2])
        acc = []
        for g in range(2):
            nc.vector.tensor_tensor(gacc[-2][g][:], gacc[-2][g][:],
                                    gacc[-1][g][:], OP.add)
            nc.vector.tensor_tensor(gacc[1][g][:], gacc[1][g][:],
                                    gacc[2][g][:], OP.add)
            nc.vector.tensor_tensor(gacc[-2][g][:], gacc[-2][g][:],
                                    gacc[0][g][:], OP.add)
            nc.vector.tensor_tensor(gacc[-2][g][:], gacc[-2][g][:],
                                    gacc[1][g][:], OP.add)
            acc.append(gacc[-2][g])

        # output projection: out[c, n] = wo.T @ head_out + bout
        for mc in range(2):
            for fc in range(NSC):
                po = psW.tile([128, 1024], F32, tag="w", name="po")
                for g in range(2):
                    nc.tensor.matmul(
                        po[:, :FC],
                        wo_t[g][:, mc * 128:(mc + 1) * 128],
                        acc[g][:, fc * FC:(fc + 1) * FC],
                        start=(g == 0), stop=(g == 1))
                ot = outp.tile([128, FC], F32, tag="ot", name="ot")
                nc.vector.tensor_scalar(ot[:], po[:, :FC], bout_t[mc][:],
                                        None, OP.add)
                nc.sync.dma_start(
                    d["out"][mc * 128:(mc + 1) * 128,
                             q0 + fc * FC:q0 + (fc + 1) * FC], ot[:])


def prep_inputs(x, w_off, b_off, w_attn, b_attn, w_val, b_val, w_out, b_out):
    """Host-side sharding: returns list of 8 per-core input dicts."""
    sel0, sel1, selp, xmask = _consts()
    # woa layout [C, 224]: 32 leading zero cols, then per q in (off_y, off_x,
    # attn): w_q at cols 32+64q:64+64q, zeros at 64+64q:96+64q.
    woa = np.zeros((C, 224), np.float16)
    wq = [w_off[1::2], w_off[0::2], w_attn]
    bq = [b_off[1::2], b_off[0::2], b_attn]
    for q in range(3):
        woa[:, 32 + 64 * q:64 + 64 * q] = wq[q].T
    boa = np.zeros((3 * 128, 1), np.float32)
    for q in range(3):
        boa[q * 128:(q + 1) * 128, 0] = np.tile(bq[q], 4)
    shared = {
        "wv": np.ascontiguousarray(w_val.T).astype(np.float32),
        "woa": woa,
        "wo": np.ascontiguousarray(w_out.T).astype(np.float16),
        "bval": np.ascontiguousarray(b_val[:, None]).astype(np.float32),
        "boa": boa,
        "bout": np.ascontiguousarray(b_out[:, None]).astype(np.float32),
        "sel0": sel0, "sel1": sel1, "selp": selp, "xmask": xmask,
        "cb": np.tile(np.array([2.0, 1.0, 0.0, -1.0, -2.0], np.float32),
                      (128, 1)),
    }
    in_maps = []
    for core in range(8):
        b, j = divmod(core, 4)
        n0 = j * NB
        xb = x[b].reshape(C, N)
        xt = np.zeros((C, XEXT), np.float32)
        lo, hi = n0 - HALO, n0 + NB + HALO
        clo, chi = max(lo, 0), min(hi, N)
        xt[:, clo - lo:chi - lo] = xb[:, clo:chi]
        vm = np.ones((128, 2 * HALO), np.float16)
        if j == 0:
            vm[:, :HALO] = 0.0
        if j == 3:
            vm[:, HALO:] = 0.0
        m = dict(shared)
        m["xT"] = xt
        m["vmask"] = vm
        in_maps.append(m)
    return in_maps


def assemble(results):
    out = np.zeros((B, C, N), np.float32)
    for core in range(8):
        b, j = divmod(core, 4)
        out[b, :, j * NB:(j + 1) * NB] = results[core]["out"]
    return out.reshape(B, C, Z, HH, WW)


last_exec_ns = None


def kernel(**inputs):
    global last_exec_ns
    from concourse.bass_utils import run_bass_kernel_spmd
    nc = build_program()
    in_maps = prep_inputs(**inputs)
    res = run_bass_kernel_spmd(nc, in_maps, list(range(8)))
    last_exec_ns = res.exec_time_ns
    return assemble(res.results)


# revision 14
# speedup vs baseline: 1.5831x; 1.0050x over previous
"""Deformable-attention-3D Trainium2 kernel (v2, fp16 MAC).

Sharding: 8 cores = (batch b in {0,1}) x (query-block j in {0..3}).
Each core handles NB=8192 queries of one batch; inputs are host-sliced
per core (full x channel-major slab + halo), outputs host-concatenated.
No collectives.

Per-core layout: partition = (head, dh) for value/output, free axis = query n.
The bilinear gather becomes 25 free-axis-offset FMAs (cells sy,sx in
[-2,2]^2) weighted by a tent-product weight grid:
    w_cell(n,h) = sum_p attn[n,h,p] * relu(1-|off_y-sy|) * relu(1-|off_x-sx|)
with x-border masks and zeroed V halo for y/batch borders.

v2 changes vs v1:
 - value tensor, tent weights, replicated weight grids, tmp and acc all
   fp16 -> DVE tensor_tensor runs in 2x_1p mode (2x).
 - Act engine evacuates the PE-replicated weight grid PSUM->SBUF fp16,
   so the big MAC multiplies are all-SBUF fp16.
 - offsets/attn projections matmul directly into packed partition rows
   of [128,*] PSUM tiles (no SBUF->SBUF repack DMAs); rows 96:128 are
   written via a 64-row accumulating matmul pair at base 64.
 - MAC mult/add run as single [128,2048]-wide ops; a tunable subset of
   cells' accumulate-adds runs on the Pool engine.
"""
import os
import numpy as np
from contextlib import ExitStack

import concourse.bass as bass
import concourse.mybir as mybir
import concourse.tile as tile

F32 = mybir.dt.float32
F32R = mybir.dt.float32r
F16 = mybir.dt.float16
AF = mybir.ActivationFunctionType
OP = mybir.AluOpType

# problem constants
B, C, Z, HH, WW = 2, 256, 16, 32, 64
H, W = HH * Z, WW          # 512, 64
N = H * W                  # 32768 queries per batch
HEADS, P, DH = 8, 4, 32
NB = N // 4                # 8192 queries per core
HALO = 192                 # V halo each side (needs >= 130)
XEXT = NB + 2 * HALO       # 8576
NSC = 4                    # superchunks per core
SC = NB // NSC             # 2048
FC = 512                   # matmul moving-dim chunk
SYS = (-2, -1, 0, 1, 2)
SXS = (-2, -1, 0, 1, 2)

# tunable: sy-groups whose accumulate-add chains run on Pool instead of DVE.
POOL_SYS = frozenset((1, 2))

_cache = {}


def _consts():
    """Host-computed constant tensors shared by all cores."""
    # selector: maps (h,p) rows -> (h', d) cols for head-group g; stacked x4
    # for the 4 base partitions (r-major packing of n-subchunks).
    # Layout [128, 256]: cols 0:128 = sel32 stacked x4 (slices r=0,1,2 used at
    # bases 0/32/64); cols 128:256 = K=64 variant for r=3 at base 64 (rows
    # 64:96 zero so the r2 partitions contribute nothing, rows 96:128 = sel32).
    sels = []
    for g in range(2):
        s = np.zeros((32, 128), np.float16)
        for h in range(8):
            for p in range(P):
                for hh in range(4):
                    if h == 4 * g + hh:
                        s[h * 4 + p, hh * 32:(hh + 1) * 32] = 1.0
        full = np.zeros((128, 256), np.float16)
        full[:, :128] = np.tile(s, (4, 1))
        full[96:128, 128:256] = s
        sels.append(full)
    # softmax p-sum selector, block-diagonal over the 4 r-blocks:
    # [128, 128], out row (r,h,p) = sum_{p'} in row (r,h,p')
    selp = np.zeros((32, 32), np.float16)
    for h in range(8):
        selp[h * 4:(h + 1) * 4, h * 4:(h + 1) * 4] = 1.0
    selp128 = np.zeros((128, 128), np.float16)
    for r in range(4):
        selp128[32 * r:32 * (r + 1), 32 * r:32 * (r + 1)] = selp
    # x-border masks for sx in (-2,-1,1,2): [128, 4*FC]
    xm = np.zeros((128, 4 * FC), np.float16)
    for k, sx in enumerate((-2, -1, 1, 2)):
        i = np.arange(FC)
        valid = ((i % W) + sx >= 0) & ((i % W) + sx < W)
        xm[:, k * FC:(k + 1) * FC] = valid.astype(np.float16)[None, :]
    return sels[0], sels[1], selp128, xm


def build_program(reps=1):
    key = ("nc", reps)
    if key in _cache:
        return _cache[key]
    nc = bass.Bass()
    d = {}
    d["xT"] = nc.dram_tensor("xT", [C, XEXT], F32, kind="ExternalInput").ap()
    d["wv"] = nc.dram_tensor("wv", [C, C], F32, kind="ExternalInput").ap()
    d["woa"] = nc.dram_tensor("woa", [C, 224], F16, kind="ExternalInput").ap()
    d["wo"] = nc.dram_tensor("wo", [C, C], F16, kind="ExternalInput").ap()
    d["bval"] = nc.dram_tensor("bval", [C, 1], F32, kind="ExternalInput").ap()
    d["boa"] = nc.dram_tensor("boa", [3 * 128, 1], F32, kind="ExternalInput").ap()
    d["bout"] = nc.dram_tensor("bout", [C, 1], F32, kind="ExternalInput").ap()
    d["sel0"] = nc.dram_tensor("sel0", [128, 256], F16, kind="ExternalInput").ap()
    d["sel1"] = nc.dram_tensor("sel1", [128, 256], F16, kind="ExternalInput").ap()
    d["selp"] = nc.dram_tensor("selp", [128, 128], F16, kind="ExternalInput").ap()
    d["xmask"] = nc.dram_tensor("xmask", [128, 4 * FC], F16, kind="ExternalInput").ap()
    d["vmask"] = nc.dram_tensor("vmask", [128, 2 * HALO], F16, kind="ExternalInput").ap()
    d["cb"] = nc.dram_tensor("cb", [128, 5], F32, kind="ExternalInput").ap()
    d["out"] = nc.dram_tensor("out", [C, NB], F32, kind="ExternalOutput").ap()
    with tile.TileContext(nc) as tc, ExitStack() as ctx, \
            nc.allow_low_precision(reason="fp16 MAC validated vs reference"):
        if reps == 1:
            _kernel_body(ctx, tc, d)
        else:
            with tc.For_i(0, reps, 1):
                _kernel_body(ctx, tc, d)
    _legalize_waits(nc)
    _cache[key] = nc
    return nc


def _legalize_waits(nc):
    """This toolchain's walrus rejects >1 sem-wait on HW-decoded structs
    (fp32-family matmuls, drains) and has small caps elsewhere. Split excess
    waits onto standalone EventSemaphore instructions placed just before the
    offender on the same engine — semantically identical (waits still
    happen-before, same order)."""
    PE = mybir.EngineType.PE
    nid = 0
    for f in nc.m.functions:
        for bb in f.blocks:
            insts = bb.instructions
            i = 0
            while i < len(insts):
                inst = insts[i]
                si = inst.sync_info
                waits = list(si.on_wait) if (si and si.on_wait) else []
                cls = type(inst).__name__
                limit = 1
                if len(waits) > limit:
                    keep = waits[len(waits) - limit:]
                    excess = waits[:len(waits) - limit]
                    inst.sync_info = mybir.SyncInfo(
                        on_wait=keep, on_update=list(si.on_update or []))
                    for w in excess:
                        ws = mybir.InstEventSemaphore(
                            name=f"WSPLIT-{nid}", ins=[], outs=[],
                            sync_info=mybir.SyncInfo(on_wait=[w], on_update=[]))
                        nid += 1
                        ws.engine = inst.engine
                        nc.register_instruction(ws, overwrite=True)
                        insts.insert(i, ws)
                        i += 1
                i += 1


def _kernel_body(ctx, tc, d):
    nc = tc.nc
    const = ctx.enter_context(tc.tile_pool(name="const", bufs=1))
    vpool = ctx.enter_context(tc.tile_pool(name="vpool", bufs=1))
    xin = ctx.enter_context(tc.tile_pool(name="xin", bufs=2))
    small = ctx.enter_context(tc.tile_pool(name="small", bufs=2))
    build = ctx.enter_context(tc.tile_pool(name="build", bufs=1))
    wgt = ctx.enter_context(tc.tile_pool(name="wgt", bufs=2))
    macp = ctx.enter_context(tc.tile_pool(name="macp", bufs=3))
    accp = ctx.enter_context(tc.tile_pool(name="accp", bufs=1))
    outp = ctx.enter_context(tc.tile_pool(name="outp", bufs=2))
    # PSUM budget (8 banks): psW 2 bufs x [128,1024] = 4, psQ 3 x [128,512]
    # = 3, psS 1 x [128,512] = 1.
    psW = ctx.enter_context(tc.tile_pool(name="psW", bufs=2, space="PSUM"))
    psQ = ctx.enter_context(tc.tile_pool(name="psQ", bufs=1, space="PSUM"))
    psS = ctx.enter_context(tc.tile_pool(name="psS", bufs=1, space="PSUM"))

    # ---- constants to SBUF
    wv_t = [const.tile([128, C], F32R, tag=f"wv{k}", name=f"wv{k}") for k in range(2)]
    wo_t = [const.tile([128, C], F16, tag=f"wo{k}", name=f"wo{k}") for k in range(2)]
    woa_t = [const.tile([128, 224], F16, tag=f"woa{k}", name=f"woa{k}") for k in range(2)]
    sel_t = [const.tile([128, 256], F16, tag=f"sel{g}", name=f"sel{g}") for g in range(2)]
    selp_t = const.tile([128, 128], F16, tag="selp", name="selp")
    xm_t = const.tile([128, 4 * FC], F16, tag="xm", name="xm")
    vm_t = const.tile([128, 2 * HALO], F16, tag="vm", name="vm")
    bval_t = [const.tile([128, 1], F32, tag=f"bv{g}", name=f"bv{g}") for g in range(2)]
    boa_t = [const.tile([128, 1], F32, tag=f"boa{q}", name=f"boa{q}") for q in range(3)]
    bout_t = [const.tile([128, 1], F32, tag=f"bo{m}", name=f"bo{m}") for m in range(2)]
    cb_t = const.tile([128, 5], F32, tag="cb", name="cb")
    for k in range(2):
        nc.sync.dma_start(wv_t[k][:], d["wv"][k * 128:(k + 1) * 128, :].bitcast(F32R))
        nc.sync.dma_start(wo_t[k][:], d["wo"][k * 128:(k + 1) * 128, :])
        nc.sync.dma_start(woa_t[k][:], d["woa"][k * 128:(k + 1) * 128, :])
        nc.sync.dma_start(bval_t[k][:], d["bval"][k * 128:(k + 1) * 128, :])
        nc.sync.dma_start(bout_t[k][:], d["bout"][k * 128:(k + 1) * 128, :])
    nc.sync.dma_start(sel_t[0][:], d["sel0"][:])
    nc.sync.dma_start(sel_t[1][:], d["sel1"][:])
    nc.sync.dma_start(selp_t[:], d["selp"][:])
    nc.sync.dma_start(xm_t[:], d["xmask"][:])
    nc.sync.dma_start(vm_t[:], d["vmask"][:])
    nc.sync.dma_start(cb_t[:], d["cb"][:])
    for q in range(3):
        nc.sync.dma_start(boa_t[q][:], d["boa"][q * 128:(q + 1) * 128, :])

    # ---- phase 1: value projection V[hg][128, XEXT], fp16, hd-major partitions
    V = [vpool.tile([128, XEXT], F16, tag=f"V{g}", name=f"V{g}") for g in range(2)]
    CH = XEXT // 8  # 1072
    for r in range(8):
        xt = [xin.tile([128, CH], F32R, tag=f"x1_{k}", name=f"x1_{k}") for k in range(2)]
        for k in range(2):
            nc.sync.dma_start(xt[k][:], d["xT"][k * 128:(k + 1) * 128,
                                                r * CH:(r + 1) * CH].bitcast(F32R))
        for g in range(2):
            nf = 0
            while nf < CH:
                f = min(FC, CH - nf)
                pv = psW.tile([128, 1024], F32, tag="w", name="pv")
                for k in range(2):
                    nc.tensor.matmul(
                        pv[:, :f],
                        wv_t[k][:, g * 128:(g + 1) * 128],
                        xt[k][:, nf:nf + f],
                        start=(k == 0), stop=(k == 1))
                nc.scalar.activation(V[g][:, r * CH + nf:r * CH + nf + f],
                                     pv[:, :f], AF.Identity,
                                     bias=bval_t[g][:], scale=1.0)
                nf += f
    # zero out-of-batch halo (vmask is all-ones for interior cores)
    for g in range(2):
        nc.vector.tensor_tensor(V[g][:, 0:HALO], V[g][:, 0:HALO],
                                vm_t[:, 0:HALO], OP.mult)
        nc.vector.tensor_tensor(V[g][:, XEXT - HALO:XEXT],
                                V[g][:, XEXT - HALO:XEXT],
                                vm_t[:, HALO:2 * HALO], OP.mult)

    # ---- phase 2: per superchunk
    for sc in range(NSC):
        q0 = sc * SC

        # offsets/attn projections straight into packed (4r x 32hp) PSUM rows.
        # rows 96:128 can't be a matmul base, so r=2,3 are a 64-row pair at
        # base 64 with zero-padded lhs columns (woa layout: [C,224], q-block
        # at cols 32+64q:64+64q, zeros elsewhere).
        pq3 = [psQ.tile([128, FC], F32, tag=f"q{q}", name=f"q{q}") for q in range(3)]
        for r in range(NSC):
            xt = [xin.tile([128, FC], F32, tag=f"x2_{k}", name=f"x2_{k}")
                  for k in range(2)]
            xt16 = [xin.tile([128, FC], F16, tag=f"x16_{k}", name=f"x16_{k}")
                    for k in range(2)]
            for k in range(2):
                nc.sync.dma_start(
                    xt[k][:],
                    d["xT"][k * 128:(k + 1) * 128,
                            HALO + q0 + r * FC:HALO + q0 + (r + 1) * FC])
                nc.gpsimd.tensor_scalar(xt16[k][:], xt[k][:], 1.0, None,
                                        OP.mult)
            for q in range(3):
                if r < 2:
                    for k in range(2):
                        nc.tensor.matmul(
                            pq3[q][32 * r:32 * (r + 1), :],
                            woa_t[k][:, 32 + 64 * q:64 + 64 * q],
                            xt16[k][:],
                            start=(k == 0), stop=(k == 1))
                elif r == 2:
                    for k in range(2):
                        nc.tensor.matmul(
                            pq3[q][64:128, :],
                            woa_t[k][:, 32 + 64 * q:96 + 64 * q],
                            xt16[k][:],
                            start=(k == 0), stop=False)
                else:
                    for k in range(2):
                        nc.tensor.matmul(
                            pq3[q][64:128, :],
                            woa_t[k][:, 64 * q:64 + 64 * q],
                            xt16[k][:],
                            start=False, stop=(k == 1))

        # biases + softmax (fp16 outputs)
        oyp = build.tile([128, FC], F32, tag="oyp", name="oyp")
        oxp = build.tile([128, FC], F32, tag="oxp", name="oxp")
        ex = build.tile([128, FC], F16, tag="ex", name="ex")
        rc = build.tile([128, FC], F16, tag="rc", name="rc")
        atp = build.tile([128, FC], F16, tag="atp", name="atp")
        nc.scalar.activation(oyp[:], pq3[0][:], AF.Identity, bias=boa_t[0][:],
                             scale=1.0)
        nc.scalar.activation(oxp[:], pq3[1][:], AF.Identity, bias=boa_t[1][:],
                             scale=1.0)
        nc.scalar.activation(ex[:], pq3[2][:], AF.Exp, bias=boa_t[2][:],
                             scale=1.0)
        pss = psS.tile([128, FC], F32, tag="ps", name="ps")
        nc.tensor.matmul(pss[:], selp_t[:], ex[:], start=True, stop=True)
        nc.vector.reciprocal(rc[:], pss[:])
        nc.vector.tensor_tensor(atp[:], ex[:], rc[:], OP.mult)

        # tent weights: AYA[sy] = attn * relu(1-|off_y - sy|),
        #               AXM[sx] = relu(1-|off_x - sx|) * xmask   (all fp16)
        aya, axm = {}, {}
        for sy in SYS:
            u = build.tile([128, FC], F32, tag="u", name="u", bufs=2)
            nc.scalar.activation(u[:], oyp[:], AF.Abs,
                                 bias=cb_t[:, sy + 2:sy + 3], scale=1.0)
            t = build.tile([128, FC], F16, tag=f"aya{sy}", name=f"aya{sy}")
            nc.scalar.activation(t[:], u[:], AF.Relu, bias=1.0, scale=-1.0)
            nc.vector.tensor_tensor(t[:], t[:], atp[:], OP.mult)
            aya[sy] = t
        xmi = {-2: 0, -1: 1, 1: 2, 2: 3}
        for sx in SXS:
            u = build.tile([128, FC], F32, tag="u", name="u", bufs=2)
            nc.scalar.activation(u[:], oxp[:], AF.Abs,
                                 bias=cb_t[:, sx + 2:sx + 3], scale=1.0)
            t = build.tile([128, FC], F16, tag=f"axm{sx}", name=f"axm{sx}")
            nc.scalar.activation(t[:], u[:], AF.Relu, bias=1.0, scale=-1.0)
            if sx != 0:
                k = xmi[sx]
                nc.vector.tensor_tensor(t[:], t[:],
                                        xm_t[:, k * FC:(k + 1) * FC], OP.mult)
            axm[sx] = t

        # cell loop: weight grid -> PE replication -> Act psum->sbuf fp16 ->
        # wide fp16 mult (DVE) + tree accumulate: one independent chain per
        # (sy-group, g); POOL_SYS groups' chains run on Pool (DVE produces
        # their tmps ahead); 4 final merge adds per g on DVE.
        gacc = {sy: {} for sy in SYS}
        for sy in sorted(SYS, key=lambda s: (s not in POOL_SYS, s)):
            on_pool = sy in POOL_SYS
            for sx in SXS:
                pr = wgt.tile([128, FC], F16, tag="pr", name="pr", bufs=3)
                nc.vector.tensor_tensor(pr[:], aya[sy][:], axm[sx][:], OP.mult)
                dlt = sy * W + sx
                vb = HALO + q0 + dlt
                first = (sx == SXS[0])
                for g in range(2):
                    wrep = wgt.tile([128, SC], F16, tag=f"wr{g}",
                                    name=f"wr{g}", bufs=2)
                    for hf in range(2):
                        pw = psW.tile([128, 1024], F32, tag="w", name="pw")
                        for rr in range(2):
                            r = 2 * hf + rr
                            if r < 3:
                                lhs = sel_t[g][32 * r:32 * (r + 1), 0:128]
                                rh = pr[32 * r:32 * (r + 1), :]
                            else:
                                lhs = sel_t[g][64:128, 128:256]
                                rh = pr[64:128, :]
                            nc.tensor.matmul(
                                pw[:, rr * FC:(rr + 1) * FC],
                                lhs, rh,
                                start=True, stop=True)
                        nc.scalar.activation(
                            wrep[:, hf * 1024:(hf + 1) * 1024], pw[:],
                            AF.Identity)
                    vs = V[g][:, vb:vb + SC]
                    if first:
                        ga = accp.tile([128, SC], F16, tag=f"ga{sy}_{g}",
                                       name=f"ga{sy}_{g}")
                        gacc[sy][g] = ga
                        nc.vector.tensor_tensor(ga[:], vs, wrep[:], OP.mult)
                    else:
                        tmp = macp.tile([128, SC], F16, tag="tmp", name="tmp",
                                        bufs=4)
                        nc.vector.tensor_tensor(tmp[:], vs, wrep[:], OP.mult)
                        if on_pool:
                            nc.gpsimd.tensor_tensor(gacc[sy][g][:],
                                                    gacc[sy][g][:], tmp[:],
                                                    OP.add)
                        else:
                            nc.vector.tensor_tensor(gacc[sy][g][:],
                                                    gacc[sy][g][:], tmp[:],
                                                    OP.add)
        # final merges per g (DVE): ((g[-2]+g[-1]) + g[0]) + (g[1] + g[2])
        acc = []
        for g in range(2):
            nc.vector.tensor_tensor(gacc[-2][g][:], gacc[-2][g][:],
                                    gacc[-1][g][:], OP.add)
            nc.vector.tensor_tensor(gacc[1][g][:], gacc[1][g][:],
                                    gacc[2][g][:], OP.add)
            nc.vector.tensor_tensor(gacc[-2][g][:], gacc[-2][g][:],
                                    gacc[0][g][:], OP.add)
            nc.vector.tensor_tensor(gacc[-2][g][:], gacc[-2][g][:],
                                    gacc[1][g][:], OP.add)
            acc.append(gacc[-2][g])

        # output projection: out[c, n] = wo.T @ head_out + bout
        for mc in range(2):
            for fc in range(NSC):
                po = psW.tile([128, 1024], F32, tag="w", name="po")
                for g in range(2):
                    nc.tensor.matmul(
                        po[:, :FC],
                        wo_t[g][:, mc * 128:(mc + 1) * 128],
                        acc[g][:, fc * FC:(fc + 1) * FC],
                        start=(g == 0), stop=(g == 1))
                ot = outp.tile([128, FC], F32, tag="ot", name="ot")
                nc.vector.tensor_scalar(ot[:], po[:, :FC], bout_t[mc][:],
                                        None, OP.add)
                nc.sync.dma_start(
                    d["out"][mc * 128:(mc + 1) * 128,
                             q0 + fc * FC:q0 + (fc + 1) * FC], ot[:])


def prep_inputs(x, w_off, b_off, w_attn, b_attn, w_val, b_val, w_out, b_out):
    """Host-side sharding: returns list of 8 per-core input dicts."""
    sel0, sel1, selp, xmask = _consts()
    # woa layout [C, 224]: 32 leading zero cols, then per q in (off_y, off_x,
    # attn): w_q at cols 32+64q:64+64q, zeros at 64+64q:96+64q.
    woa = np.zeros((C, 224), np.float16)
    wq = [w_off[1::2], w_off[0::2], w_attn]
    bq = [b_off[1::2], b_off[0::2], b_attn]
    for q in range(3):
        woa[:, 32 + 64 * q:64 + 64 * q] = wq[q].T
    boa = np.zeros((3 * 128, 1), np.float32)
    for q in range(3):
        boa[q * 128:(q + 1) * 128, 0] = np.tile(bq[q], 4)
    shared = {
        "wv": np.ascontiguousarray(w_val.T).astype(np.float32),
        "woa": woa,
        "wo": np.ascontiguousarray(w_out.T).astype(np.float16),
        "bval": np.ascontiguousarray(b_val[:, None]).astype(np.float32),
        "boa": boa,
        "bout": np.ascontiguousarray(b_out[:, None]).astype(np.float32),
        "sel0": sel0, "sel1": sel1, "selp": selp, "xmask": xmask,
        "cb": np.tile(np.array([2.0, 1.0, 0.0, -1.0, -2.0], np.float32),
                      (128, 1)),
    }
    in_maps = []
    for core in range(8):
        b, j = divmod(core, 4)
        n0 = j * NB
        xb = x[b].reshape(C, N)
        xt = np.zeros((C, XEXT), np.float32)
        lo, hi = n0 - HALO, n0 + NB + HALO
        clo, chi = max(lo, 0), min(hi, N)
        xt[:, clo - lo:chi - lo] = xb[:, clo:chi]
        vm = np.ones((128, 2 * HALO), np.float16)
        if j == 0:
            vm[:, :HALO] = 0.0
        if j == 3:
            vm[:, HALO:] = 0.0
        m = dict(shared)
        m["xT"] = xt
        m["vmask"] = vm
        in_maps.append(m)
    return in_maps


def assemble(results):
    out = np.zeros((B, C, N), np.float32)
    for core in range(8):
        b, j = divmod(core, 4)
        out[b, :, j * NB:(j + 1) * NB] = results[core]["out"]
    return out.reshape(B, C, Z, HH, WW)


last_exec_ns = None


def kernel(**inputs):
    global last_exec_ns
    from concourse.bass_utils import run_bass_kernel_spmd
    nc = build_program()
    in_maps = prep_inputs(**inputs)
    res = run_bass_kernel_spmd(nc, in_maps, list(range(8)))
    last_exec_ns = res.exec_time_ns
    return assemble(res.results)
